# revision 20
# baseline (speedup 1.0000x reference)
"""Trainium2 Bass kernel for EEGToLatentGCN.

Math: because the reference stacks all B*C nodes but uses a single 17-node
edge_index, message passing only ever touches global nodes 0..16 (= batch
element 0). Every other node goes through a plain per-node MLP:
    h = leaky(x @ We + be); h = leaky(h @ W1 + b1); h = leaky(h @ W2 + b2);
    h = leaky(h @ W3 + b3); g = mean_17(h); out = leaky(g @ Wh1 + bh1) @ Wh2 + bh2
The device kernel computes that MLP for all graphs (data-parallel over 8
cores); batch element 0 (17 nodes) is recomputed exactly on the host with the
real graph propagation and overwrites out[0].

Device layout (v2, bf16):
- x is pre-transposed on the host to feature-major [81, R] bf16 with row 80
  all-ones, so no PE transposes are needed and the embedding bias rides in
  the matmul as an 81st contraction row (same trick gives conv1 its bias via
  a 65th ones row in h0). That makes the embed/conv1 PSUM drains a single
  scalar_tensor_tensor leaky (max(0.01*u, u)) on DVE/Pool.
- conv2/conv3 drains run on ACT (fused bias+Lrelu, one op per chunk).
- 17-node mean pool: one DVE tensor_reduce per macro-tile into a persistent
  bf16 gt tile (1/17 folded into Wh1).
- head: leaky(g@Wh1+bh1)@Wh2+bh2 with the final bias added by DVE/Pool
  tensor_tensor during the PSUM->SBUF copy (no bias matmuls); head stages are
  emitted interleaved ("pumped") between conv stages so PE never idles on
  head PSUM dependencies.
"""

import numpy as np
import ml_dtypes

import concourse.bass as bass
import concourse.mybir as mybir
import concourse.tile as tile
from concourse import bacc
from concourse.bass_utils import run_bass_kernel_spmd

F32 = mybir.dt.float32
BF16 = mybir.dt.bfloat16
LRELU = mybir.ActivationFunctionType.Lrelu
ADD = mybir.AluOpType.add
MULT = mybir.AluOpType.mult
MAX = mybir.AluOpType.max
AX_X = mybir.AxisListType.X

NCORES = 8
B, C, T, F, H, L = 16384, 17, 80, 64, 256, 1024
BS = B // NCORES      # graphs per core
R = BS * C            # node rows per core
MT_G = 30             # graphs per macro-tile
MT_R = MT_G * C       # 510 rows
N_MT = BS // MT_G     # 68 full macro-tiles
TAIL_G = BS - MT_G * N_MT  # 8
SEG_MTS = 8           # macro-tiles per x DMA segment
HT_G = 256            # graphs per head tile
SLOPE = 0.01
BF = ml_dtypes.bfloat16

_CACHE = {}


def _leaky_np(v):
    return np.where(v > 0, v, SLOPE * v)


def _build(reps=1):
    nc = bacc.Bacc("TRN2", target_bir_lowering=False, debug=False)

    x_p = nc.declare_dram_parameter("x", [81, R], BF16, isOutput=False)
    wemb_p = nc.declare_dram_parameter("wemb", [81, F], BF16, isOutput=False)
    w1_p = nc.declare_dram_parameter("w1", [F + 1, 2, 128], BF16, isOutput=False)
    w2_p = nc.declare_dram_parameter("w2", [128, 2, H], BF16, isOutput=False)
    b2_p = nc.declare_dram_parameter("b2", [128, 2], F32, isOutput=False)
    w3_p = nc.declare_dram_parameter("w3", [128, 2, H], BF16, isOutput=False)
    b3_p = nc.declare_dram_parameter("b3", [128, 2], F32, isOutput=False)
    wh1_p = nc.declare_dram_parameter("wh1", [128, 2, H], BF16, isOutput=False)
    bh1_p = nc.declare_dram_parameter("bh1", [128, 2], F32, isOutput=False)
    wh2_p = nc.declare_dram_parameter("wh2", [128, 2, L], BF16, isOutput=False)
    bh2r_p = nc.declare_dram_parameter("bh2r", [128, L], F32, isOutput=False)
    out_p = nc.declare_dram_parameter("out", [BS, L], F32, isOutput=True)

    # x DMA segments: a small first segment (1 MT) so the pipeline starts
    # fast, then 8 of SEG_MTS*510 columns, then the remainder.
    seg_cols = [MT_R] + [SEG_MTS * MT_R] * 8
    seg_cols.append(R - sum(seg_cols))

    def mt_seg(mt):
        # -> (segment index, column offset within segment)
        if mt == 0:
            return 0, 0
        if mt <= 64:
            return 1 + (mt - 1) // SEG_MTS, ((mt - 1) % SEG_MTS) * MT_R
        return 9, (mt - 65) * MT_R

    with tile.TileContext(nc) as tc:
        with tc.tile_pool(name="consts", bufs=1) as consts:
            wemb_t = consts.tile([81, F], BF16)
            w1_t = consts.tile([F + 1, 2, 128], BF16)
            w2_t = consts.tile([128, 2, H], BF16)
            b2_t = consts.tile([128, 2], F32)
            w3_t = consts.tile([128, 2, H], BF16)
            b3_t = consts.tile([128, 2], F32)
            wh1_t = consts.tile([128, 2, H], BF16)
            bh1_t = consts.tile([128, 2], F32)
            wh2_t = consts.tile([128, 2, L], BF16)
            bh2r_t = consts.tile([128, L], F32)
            xsegs = [consts.tile([81, n], BF16, name=f"xseg{i}")
                     for i, n in enumerate(seg_cols)]
            gt = consts.tile([128, 2, BS], BF16)

            # DMA order: what MT0 needs first, head weights early enough for
            # ht0 (~MT9), x segments k needed at MT 8k.
            nc.sync.dma_start(wemb_t[:], wemb_p[:])
            nc.sync.dma_start(w1_t[:], w1_p[:])
            off = 0
            nc.sync.dma_start(xsegs[0][:], x_p[:, off:off + seg_cols[0]])
            off += seg_cols[0]
            nc.sync.dma_start(w2_t[:], w2_p[:])
            nc.sync.dma_start(b2_t[:], b2_p[:])
            nc.sync.dma_start(w3_t[:], w3_p[:])
            nc.sync.dma_start(b3_t[:], b3_p[:])
            nc.sync.dma_start(xsegs[1][:], x_p[:, off:off + seg_cols[1]])
            off += seg_cols[1]
            nc.sync.dma_start(wh1_t[:], wh1_p[:])
            nc.sync.dma_start(bh1_t[:], bh1_p[:])
            nc.sync.dma_start(wh2_t[:], wh2_p[:])
            nc.sync.dma_start(bh2r_t[:], bh2r_p[:])
            for s in range(2, len(seg_cols)):
                nc.sync.dma_start(xsegs[s][:], x_p[:, off:off + seg_cols[s]])
                off += seg_cols[s]

            for _rep in range(reps):
              with tc.tile_pool(name="h0p", bufs=3) as h0p, \
                 tc.tile_pool(name="mwork", bufs=3) as spool, \
                 tc.tile_pool(name="ps_e", bufs=1, space="PSUM") as pse_pool, \
                 tc.tile_pool(name="ps_c1", bufs=1, space="PSUM") as psc1_pool, \
                 tc.tile_pool(name="ps_b", bufs=5, space="PSUM") as psb_pool, \
                 tc.tile_pool(name="hwork", bufs=2) as hpool:

                # pre-set the ones row (bias trick) in each rotating h0 buffer
                for _ in range(3):
                    t = h0p.tile([F + 1, MT_R], BF16, tag="h0")
                    nc.gpsimd.memset(t[F:F + 1, :], 1.0)

                pending = []

                def pump(n=1):
                    for _ in range(n):
                        if pending:
                            pending.pop(0)()

                def make_A(ht, c, g1):
                    def t():
                        g0 = ht * HT_G
                        psf = psb_pool.tile([128, 512], F32, tag="psb")
                        ps = psf[:, 0:HT_G]
                        nc.tensor.matmul(ps[:],
                                         wh1_t[:, 0, c * 128:(c + 1) * 128],
                                         gt[:, 0, g0:g0 + HT_G],
                                         start=True, stop=False)
                        nc.tensor.matmul(ps[:],
                                         wh1_t[:, 1, c * 128:(c + 1) * 128],
                                         gt[:, 1, g0:g0 + HT_G],
                                         start=False, stop=True)
                        u = hpool.tile([128, HT_G], BF16, tag=f"hu{c}")
                        nc.vector.tensor_scalar(u[:], ps[:],
                                                bh1_t[:, c:c + 1], None,
                                                op0=ADD)
                        nc.vector.scalar_tensor_tensor(
                            g1[:, c, :], u[:], SLOPE, u[:], op0=MULT, op1=MAX)
                    return t

                def make_B(ht, m, nb, g1, o_sb):
                    def t():
                        nsl = slice(nb * 512, (nb + 1) * 512)
                        pso = psb_pool.tile([128, 512], F32, tag="psb")
                        nc.tensor.matmul(pso[:],
                                         g1[:, 0, m * 128:(m + 1) * 128],
                                         wh2_t[:, 0, nsl],
                                         start=True, stop=False)
                        nc.tensor.matmul(pso[:],
                                         g1[:, 1, m * 128:(m + 1) * 128],
                                         wh2_t[:, 1, nsl],
                                         start=False, stop=True)
                        nc.vector.tensor_tensor(o_sb[:, m, nsl], pso[:],
                                                bh2r_t[:, nsl], op=ADD)
                        if m == 1 and nb == 1:
                            g0 = ht * HT_G
                            nc.sync.dma_start(
                                out_p[g0:g0 + HT_G, :].rearrange(
                                    "(j p) l -> p j l", p=128),
                                o_sb[:])
                    return t

                def enqueue_head(ht):
                    g1 = hpool.tile([128, 2, HT_G], BF16, tag="g1")
                    o_sb = hpool.tile([128, 2, L], F32, tag="osb")
                    pending.append(make_A(ht, 0, g1))
                    pending.append(make_A(ht, 1, g1))
                    for m in range(2):
                        for nb in range(2):
                            pending.append(make_B(ht, m, nb, g1, o_sb))

                next_ht = 0
                graphs_done = 0
                N_TOT = N_MT + (1 if TAIL_G else 0)

                def mt_dims(mt):
                    g = MT_G if mt < N_MT else TAIL_G
                    return g, g * C

                h0s, h1s, h2s, h3s = {}, {}, {}, {}

                # software-pipelined: step t emits embed(t), conv1(t-1),
                # conv2(t-2), conv3(t-3) so every PE instruction's inputs
                # were produced a full step earlier.
                for t in range(N_TOT + 3):
                    if t < N_TOT:
                        mt = t
                        g, r = mt_dims(mt)
                        seg, off = mt_seg(mt)
                        xs = xsegs[seg]
                        # embed [80]->[64] (+bias row); leaky on DVE
                        ps_e = pse_pool.tile([F, MT_R], F32, tag="pse")
                        nc.tensor.matmul(ps_e[:, 0:r], wemb_t[:],
                                         xs[:, off:off + r],
                                         start=True, stop=True)
                        h0 = h0p.tile([F + 1, MT_R], BF16, tag="h0")
                        nc.scalar.activation(h0[0:F, 0:r], ps_e[:, 0:r],
                                             LRELU, bias=0.0, scale=1.0,
                                             alpha=SLOPE)
                        h0s[mt] = h0

                    if 1 <= t < N_TOT + 1:
                        mt = t - 1
                        g, r = mt_dims(mt)
                        h0 = h0s.pop(mt)
                        # conv1 [64+1]->[256]; leaky on Pool
                        h1 = spool.tile([128, 2, MT_R], BF16, tag="h1")
                        ps1 = psc1_pool.tile([128, 2, 512], F32, tag="ps1")
                        for c in range(2):
                            nc.tensor.matmul(ps1[:, c, 0:r], w1_t[:, c, :],
                                             h0[:, 0:r], start=True, stop=True)
                        nc.scalar.activation(h1[:, :, 0:r], ps1[:, :, 0:r],
                                             LRELU, bias=0.0, scale=1.0,
                                             alpha=SLOPE)
                        h1s[mt] = h1
                    pump()

                    if 2 <= t < N_TOT + 2:
                        mt = t - 2
                        g, r = mt_dims(mt)
                        h1 = h1s.pop(mt)
                        # conv2 [256]->[256]; bias+leaky on ACT
                        h2 = spool.tile([128, 2, MT_R], BF16, tag="h2")
                        for c in range(2):
                            psf = psb_pool.tile([128, 512], F32, tag="psb")
                            ps = psf[:, 0:MT_R]
                            nc.tensor.matmul(ps[:, 0:r],
                                             w2_t[:, 0, c * 128:(c + 1) * 128],
                                             h1[:, 0, 0:r],
                                             start=True, stop=False)
                            nc.tensor.matmul(ps[:, 0:r],
                                             w2_t[:, 1, c * 128:(c + 1) * 128],
                                             h1[:, 1, 0:r],
                                             start=False, stop=True)
                            nc.scalar.activation(h2[:, c, 0:r], ps[:, 0:r],
                                                 LRELU, bias=b2_t[:, c:c + 1],
                                                 scale=1.0, alpha=SLOPE)
                        h2s[mt] = h2

                    if 3 <= t < N_TOT + 3:
                        mt = t - 3
                        g, r = mt_dims(mt)
                        g0 = mt * MT_G
                        h2 = h2s.pop(mt)
                        # conv3 [256]->[256]; bias+leaky on ACT
                        h3 = spool.tile([128, 2, MT_R], BF16, tag="h3")
                        for c in range(2):
                            psf = psb_pool.tile([128, 512], F32, tag="psb")
                            ps = psf[:, 0:MT_R]
                            nc.tensor.matmul(ps[:, 0:r],
                                             w3_t[:, 0, c * 128:(c + 1) * 128],
                                             h2[:, 0, 0:r],
                                             start=True, stop=False)
                            nc.tensor.matmul(ps[:, 0:r],
                                             w3_t[:, 1, c * 128:(c + 1) * 128],
                                             h2[:, 1, 0:r],
                                             start=False, stop=True)
                            u3 = spool.tile([128, MT_R], BF16, tag=f"u3{c}")
                            v3 = spool.tile([128, MT_R], BF16, tag=f"v3{c}")
                            nc.vector.tensor_scalar(
                                u3[:, 0:r], ps[:, 0:r], b3_t[:, c:c + 1],
                                None, op0=ADD)
                            nc.vector.tensor_scalar(
                                v3[:, 0:r], u3[:, 0:r], SLOPE, None, op0=MULT)
                            nc.vector.tensor_tensor(
                                h3[:, c, 0:r], u3[:, 0:r], v3[:, 0:r], op=MAX)
                        # mean-pool over 17 nodes (sum; 1/17 inside Wh1)
                        # as an add tree: L1-2 on DVE tensor_tensor (2x perf
                        # mode on packed bf16), L3-5 on Pool via stt-adds.
                        # k-plane stride is exactly 30*17, so (k, g) flattens
                        # into one 60-wide dim and every AP stays <= 3D.
                        if mt < N_MT:
                            hv = h3[:, :, 0:r].rearrange(
                                "p k (g s) -> p (k g) s", s=C)
                            KG = 2 * MT_G
                            t8 = spool.tile([128, KG, 8], BF16, tag="t8")
                            t4 = spool.tile([128, KG, 4], BF16, tag="t4")
                            t2 = spool.tile([128, KG, 2], BF16, tag="t2")
                            t1 = spool.tile([128, KG], BF16, tag="t1")
                            nc.gpsimd.tensor_tensor(
                                t8[:], hv[:, :, 0:8], hv[:, :, 8:16], op=ADD)
                            nc.gpsimd.tensor_tensor(
                                t4[:], t8[:, :, 0:4], t8[:, :, 4:8], op=ADD)
                            nc.gpsimd.tensor_tensor(
                                t2[:], t4[:, :, 0:2], t4[:, :, 2:4], op=ADD)
                            nc.gpsimd.tensor_tensor(
                                t1[:], t2[:, :, 0], t2[:, :, 1], op=ADD)
                            nc.gpsimd.tensor_tensor(
                                gt[:, :, g0:g0 + g],
                                t1[:].rearrange("p (k g) -> p k g", k=2),
                                hv[:, :, 16].rearrange(
                                    "p (k g) -> p k g", k=2),
                                op=ADD)
                        else:
                            for k in range(2):
                                hvk = h3[:, k, 0:r].rearrange(
                                    "p (g s) -> p g s", s=C)
                                w8 = spool.tile([128, TAIL_G, 8], BF16,
                                                tag="w8")
                                w4 = spool.tile([128, TAIL_G, 4], BF16,
                                                tag="w4")
                                w2 = spool.tile([128, TAIL_G, 2], BF16,
                                                tag="w2")
                                w1t = spool.tile([128, TAIL_G], BF16,
                                                 tag="w1t")
                                nc.gpsimd.tensor_tensor(
                                    w8[:], hvk[:, :, 0:8], hvk[:, :, 8:16],
                                    op=ADD)
                                nc.gpsimd.tensor_tensor(
                                    w4[:], w8[:, :, 0:4], w8[:, :, 4:8],
                                    op=ADD)
                                nc.gpsimd.tensor_tensor(
                                    w2[:], w4[:, :, 0:2], w4[:, :, 2:4],
                                    op=ADD)
                                nc.gpsimd.tensor_tensor(
                                    w1t[:], w2[:, :, 0], w2[:, :, 1], op=ADD)
                                nc.gpsimd.tensor_tensor(
                                    gt[:, k, g0:g0 + g], w1t[:],
                                    hvk[:, :, 16], op=ADD)
                        # head tiles become eligible one step after
                        # their last reduce was emitted, so the gt columns
                        # are already written when the head matmuls issue
                        while (next_ht + 1) * HT_G <= graphs_done:
                            enqueue_head(next_ht)
                            next_ht += 1
                        graphs_done += g
                    pump()

                while next_ht < BS // HT_G:
                    enqueue_head(next_ht)
                    next_ht += 1
                while pending:
                    pump()

    nc.compile()
    return nc


def _get_nc(reps=1):
    key = ("nc", reps)
    if key not in _CACHE:
        _CACHE[key] = _build(reps)
    return _CACHE[key]


def prep_weights(W_emb, b_emb, W1, b1, W2, b2, W3, b3, Wh1, bh1, Wh2, bh2):
    """Host-side packing of all weight/bias tensors into device layouts."""
    def kchunks(w):
        # [256, out] -> [128, 2, out] (k-chunk as middle axis)
        return np.ascontiguousarray(
            w.reshape(2, 128, w.shape[1]).transpose(1, 0, 2))

    def bcols(b):
        # [256] -> [128, 2] fp32
        return np.ascontiguousarray(b.reshape(2, 128).T.astype(np.float32))

    wemb81 = np.vstack([W_emb, b_emb[None, :]]).astype(BF)         # [81, 64]
    w1c = np.stack(
        [np.vstack([W1[:, c * 128:(c + 1) * 128],
                    b1[None, c * 128:(c + 1) * 128]]) for c in range(2)],
        axis=1).astype(BF)                                          # [65,2,128]
    return {
        "wemb": np.ascontiguousarray(wemb81),
        "w1": np.ascontiguousarray(w1c),
        "w2": kchunks(W2).astype(BF), "b2": bcols(b2),
        "w3": kchunks(W3).astype(BF), "b3": bcols(b3),
        "wh1": kchunks(Wh1 * (1.0 / C)).astype(BF), "bh1": bcols(bh1),
        "wh2": kchunks(Wh2).astype(BF),
        "bh2r": np.ascontiguousarray(
            np.broadcast_to(bh2[None, :], (128, L)).astype(np.float32)),
    }


def prep_x(x):
    """[B, C, T] f32 -> per-core feature-major [81, R] bf16, row 80 = ones."""
    xT = np.ascontiguousarray(x.reshape(B * C, T).T)   # [80, B*C]
    per_core = []
    for i in range(NCORES):
        xc = np.empty((81, R), np.float32)
        xc[0:T] = xT[:, i * R:(i + 1) * R]
        xc[T] = 1.0
        per_core.append(xc.astype(BF))
    return per_core


def _fixup_graph0(x, W_emb, b_emb, W1, b1, W2, b2, W3, b3, Wh1, bh1, Wh2, bh2,
                  src, dst):
    """Exact recompute of batch element 0 with real GCN propagation."""
    deg = np.ones(C, np.float64)
    np.add.at(deg, dst.astype(np.int64), 1.0)
    dinv = 1.0 / np.sqrt(deg)
    A = np.zeros((C, C), np.float64)
    A[np.arange(C), np.arange(C)] = dinv * dinv
    np.add.at(A, (dst.astype(np.int64), src.astype(np.int64)),
              dinv[src.astype(np.int64)] * dinv[dst.astype(np.int64)])

    h = _leaky_np(x[0].astype(np.float64) @ W_emb + b_emb)
    for Wc, bc in [(W1, b1), (W2, b2), (W3, b3)]:
        h = _leaky_np(A @ (h @ Wc) + bc)
    g = h.mean(axis=0)
    return (_leaky_np(g @ Wh1 + bh1) @ Wh2 + bh2).astype(np.float32)


def kernel(x, W_emb, b_emb, W1, b1, W2, b2, W3, b3, Wh1, bh1, Wh2, bh2,
           src, dst):
    x = np.ascontiguousarray(np.asarray(x, np.float32))
    W_emb = np.asarray(W_emb, np.float32)
    b_emb = np.asarray(b_emb, np.float32)
    W1 = np.asarray(W1, np.float32)
    b1 = np.asarray(b1, np.float32)
    W2 = np.asarray(W2, np.float32)
    b2 = np.asarray(b2, np.float32)
    W3 = np.asarray(W3, np.float32)
    b3 = np.asarray(b3, np.float32)
    Wh1 = np.asarray(Wh1, np.float32)
    bh1 = np.asarray(bh1, np.float32)
    Wh2 = np.asarray(Wh2, np.float32)
    bh2 = np.asarray(bh2, np.float32)

    weights = prep_weights(W_emb, b_emb, W1, b1, W2, b2, W3, b3,
                           Wh1, bh1, Wh2, bh2)
    xs = prep_x(x)
    in_maps = []
    for i in range(NCORES):
        m = dict(weights)
        m["x"] = xs[i]
        in_maps.append(m)

    nc = _get_nc()
    res = run_bass_kernel_spmd(nc, in_maps, core_ids=list(range(NCORES)))
    out = np.concatenate([res.results[i]["out"] for i in range(NCORES)], axis=0)

    out[0] = _fixup_graph0(x, W_emb, b_emb, W1, b1, W2, b2, W3, b3,
                           Wh1, bh1, Wh2, bh2, np.asarray(src), np.asarray(dst))
    return out


# revision 21
# speedup vs baseline: 1.0061x; 1.0061x over previous
"""Trainium2 Bass kernel for EEGToLatentGCN.

Math: because the reference stacks all B*C nodes but uses a single 17-node
edge_index, message passing only ever touches global nodes 0..16 (= batch
element 0). Every other node goes through a plain per-node MLP:
    h = leaky(x @ We + be); h = leaky(h @ W1 + b1); h = leaky(h @ W2 + b2);
    h = leaky(h @ W3 + b3); g = mean_17(h); out = leaky(g @ Wh1 + bh1) @ Wh2 + bh2
The device kernel computes that MLP for all graphs (data-parallel over 8
cores); batch element 0 (17 nodes) is recomputed exactly on the host with the
real graph propagation and overwrites out[0].

Device layout (v2, bf16):
- x is pre-transposed on the host to feature-major [81, R] bf16 with row 80
  all-ones, so no PE transposes are needed and the embedding bias rides in
  the matmul as an 81st contraction row (same trick gives conv1 its bias via
  a 65th ones row in h0). That makes the embed/conv1 PSUM drains a single
  scalar_tensor_tensor leaky (max(0.01*u, u)) on DVE/Pool.
- conv2/conv3 drains run on ACT (fused bias+Lrelu, one op per chunk).
- 17-node mean pool: one DVE tensor_reduce per macro-tile into a persistent
  bf16 gt tile (1/17 folded into Wh1).
- head: leaky(g@Wh1+bh1)@Wh2+bh2 with the final bias added by DVE/Pool
  tensor_tensor during the PSUM->SBUF copy (no bias matmuls); head stages are
  emitted interleaved ("pumped") between conv stages so PE never idles on
  head PSUM dependencies.
"""

import numpy as np
import ml_dtypes

import concourse.bass as bass
import concourse.mybir as mybir
import concourse.tile as tile
from concourse import bacc
from concourse.bass_utils import run_bass_kernel_spmd

F32 = mybir.dt.float32
BF16 = mybir.dt.bfloat16
LRELU = mybir.ActivationFunctionType.Lrelu
ADD = mybir.AluOpType.add
MULT = mybir.AluOpType.mult
MAX = mybir.AluOpType.max
AX_X = mybir.AxisListType.X

NCORES = 8
B, C, T, F, H, L = 16384, 17, 80, 64, 256, 1024
BS = B // NCORES      # graphs per core
R = BS * C            # node rows per core
MT_G = 30             # graphs per macro-tile
MT_R = MT_G * C       # 510 rows
N_MT = BS // MT_G     # 68 full macro-tiles
TAIL_G = BS - MT_G * N_MT  # 8
SEG_MTS = 8           # macro-tiles per x DMA segment
HT_G = 256            # graphs per head tile
SLOPE = 0.01
BF = ml_dtypes.bfloat16

_CACHE = {}


def _leaky_np(v):
    return np.where(v > 0, v, SLOPE * v)


def _build(reps=1):
    nc = bacc.Bacc("TRN2", target_bir_lowering=False, debug=False)

    x_p = nc.declare_dram_parameter("x", [81, R], BF16, isOutput=False)
    wemb_p = nc.declare_dram_parameter("wemb", [81, F], BF16, isOutput=False)
    w1_p = nc.declare_dram_parameter("w1", [F + 1, 2, 128], BF16, isOutput=False)
    w2_p = nc.declare_dram_parameter("w2", [128, 2, H], BF16, isOutput=False)
    b2_p = nc.declare_dram_parameter("b2", [128, 2], F32, isOutput=False)
    w3_p = nc.declare_dram_parameter("w3", [128, 2, H], BF16, isOutput=False)
    b3_p = nc.declare_dram_parameter("b3", [128, 2], F32, isOutput=False)
    wh1_p = nc.declare_dram_parameter("wh1", [128, 2, H], BF16, isOutput=False)
    bh1_p = nc.declare_dram_parameter("bh1", [128, 2], F32, isOutput=False)
    wh2_p = nc.declare_dram_parameter("wh2", [128, 2, L], BF16, isOutput=False)
    bh2r_p = nc.declare_dram_parameter("bh2r", [128, L], F32, isOutput=False)
    out_p = nc.declare_dram_parameter("out", [BS, L], F32, isOutput=True)

    # x DMA segments: a small first segment (1 MT) so the pipeline starts
    # fast, then 8 of SEG_MTS*510 columns, then the remainder.
    seg_cols = [MT_R] + [SEG_MTS * MT_R] * 8
    seg_cols.append(R - sum(seg_cols))

    def mt_seg(mt):
        # -> (segment index, column offset within segment)
        if mt == 0:
            return 0, 0
        if mt <= 64:
            return 1 + (mt - 1) // SEG_MTS, ((mt - 1) % SEG_MTS) * MT_R
        return 9, (mt - 65) * MT_R

    with tile.TileContext(nc) as tc:
        with tc.tile_pool(name="consts", bufs=1) as consts:
            wemb_t = consts.tile([81, F], BF16)
            w1_t = consts.tile([F + 1, 2, 128], BF16)
            w2_t = consts.tile([128, 2, H], BF16)
            b2_t = consts.tile([128, 2], F32)
            w3_t = consts.tile([128, 2, H], BF16)
            b3_t = consts.tile([128, 2], F32)
            wh1_t = consts.tile([128, 2, H], BF16)
            bh1_t = consts.tile([128, 2], F32)
            wh2_t = consts.tile([128, 2, L], BF16)
            bh2r_t = consts.tile([128, L], F32)
            xsegs = [consts.tile([81, n], BF16, name=f"xseg{i}")
                     for i, n in enumerate(seg_cols)]
            gt = consts.tile([128, 2, BS], BF16)

            # DMA order: what MT0 needs first, head weights early enough for
            # ht0 (~MT9), x segments k needed at MT 8k.
            nc.sync.dma_start(wemb_t[:], wemb_p[:])
            nc.sync.dma_start(w1_t[:], w1_p[:])
            off = 0
            nc.sync.dma_start(xsegs[0][:], x_p[:, off:off + seg_cols[0]])
            off += seg_cols[0]
            nc.sync.dma_start(w2_t[:], w2_p[:])
            nc.sync.dma_start(b2_t[:], b2_p[:])
            nc.sync.dma_start(w3_t[:], w3_p[:])
            nc.sync.dma_start(b3_t[:], b3_p[:])
            nc.sync.dma_start(xsegs[1][:], x_p[:, off:off + seg_cols[1]])
            off += seg_cols[1]
            nc.sync.dma_start(wh1_t[:], wh1_p[:])
            nc.sync.dma_start(bh1_t[:], bh1_p[:])
            nc.sync.dma_start(wh2_t[:], wh2_p[:])
            nc.sync.dma_start(bh2r_t[:], bh2r_p[:])
            for s in range(2, len(seg_cols)):
                nc.sync.dma_start(xsegs[s][:], x_p[:, off:off + seg_cols[s]])
                off += seg_cols[s]

            for _rep in range(reps):
              with tc.tile_pool(name="h0p", bufs=3) as h0p, \
                 tc.tile_pool(name="mwork", bufs=3) as spool, \
                 tc.tile_pool(name="ps_e", bufs=1, space="PSUM") as pse_pool, \
                 tc.tile_pool(name="ps_c1", bufs=1, space="PSUM") as psc1_pool, \
                 tc.tile_pool(name="ps_b", bufs=5, space="PSUM") as psb_pool, \
                 tc.tile_pool(name="hwork", bufs=2) as hpool:

                # pre-set the ones row (bias trick) in each rotating h0 buffer
                for _ in range(3):
                    t = h0p.tile([F + 1, MT_R], BF16, tag="h0")
                    nc.gpsimd.memset(t[F:F + 1, :], 1.0)

                pending = []

                def pump(n=1):
                    for _ in range(n):
                        if pending:
                            pending.pop(0)()

                def make_A(ht, c, g1):
                    def t():
                        g0 = ht * HT_G
                        psf = psb_pool.tile([128, 512], F32, tag="psb")
                        ps = psf[:, 0:HT_G]
                        nc.tensor.matmul(ps[:],
                                         wh1_t[:, 0, c * 128:(c + 1) * 128],
                                         gt[:, 0, g0:g0 + HT_G],
                                         start=True, stop=False)
                        nc.tensor.matmul(ps[:],
                                         wh1_t[:, 1, c * 128:(c + 1) * 128],
                                         gt[:, 1, g0:g0 + HT_G],
                                         start=False, stop=True)
                        u = hpool.tile([128, HT_G], BF16, tag=f"hu{c}")
                        nc.vector.tensor_scalar(u[:], ps[:],
                                                bh1_t[:, c:c + 1], None,
                                                op0=ADD)
                        nc.vector.scalar_tensor_tensor(
                            g1[:, c, :], u[:], SLOPE, u[:], op0=MULT, op1=MAX)
                    return t

                def make_B(ht, m, nb, g1, o_sb):
                    def t():
                        nsl = slice(nb * 512, (nb + 1) * 512)
                        pso = psb_pool.tile([128, 512], F32, tag="psb")
                        nc.tensor.matmul(pso[:],
                                         g1[:, 0, m * 128:(m + 1) * 128],
                                         wh2_t[:, 0, nsl],
                                         start=True, stop=False)
                        nc.tensor.matmul(pso[:],
                                         g1[:, 1, m * 128:(m + 1) * 128],
                                         wh2_t[:, 1, nsl],
                                         start=False, stop=True)
                        nc.vector.tensor_tensor(o_sb[:, m, nsl], pso[:],
                                                bh2r_t[:, nsl], op=ADD)
                        if nb == 1:
                            g0 = ht * HT_G + m * 128
                            nc.sync.dma_start(out_p[g0:g0 + 128, :],
                                              o_sb[:, m, :])
                    return t

                def enqueue_head(ht):
                    g1 = hpool.tile([128, 2, HT_G], BF16, tag="g1")
                    o_sb = hpool.tile([128, 2, L], F32, tag="osb")
                    pending.append(make_A(ht, 0, g1))
                    pending.append(make_A(ht, 1, g1))
                    for m in range(2):
                        for nb in range(2):
                            pending.append(make_B(ht, m, nb, g1, o_sb))

                next_ht = 0
                graphs_done = 0
                N_TOT = N_MT + (1 if TAIL_G else 0)

                def mt_dims(mt):
                    g = MT_G if mt < N_MT else TAIL_G
                    return g, g * C

                h0s, h1s, h2s, h3s = {}, {}, {}, {}

                # software-pipelined: step t emits embed(t), conv1(t-1),
                # conv2(t-2), conv3(t-3) so every PE instruction's inputs
                # were produced a full step earlier.
                for t in range(N_TOT + 3):
                    if t < N_TOT:
                        mt = t
                        g, r = mt_dims(mt)
                        seg, off = mt_seg(mt)
                        xs = xsegs[seg]
                        # embed [80]->[64] (+bias row); leaky on DVE
                        ps_e = pse_pool.tile([F, MT_R], F32, tag="pse")
                        nc.tensor.matmul(ps_e[:, 0:r], wemb_t[:],
                                         xs[:, off:off + r],
                                         start=True, stop=True)
                        h0 = h0p.tile([F + 1, MT_R], BF16, tag="h0")
                        nc.scalar.activation(h0[0:F, 0:r], ps_e[:, 0:r],
                                             LRELU, bias=0.0, scale=1.0,
                                             alpha=SLOPE)
                        h0s[mt] = h0

                    if 1 <= t < N_TOT + 1:
                        mt = t - 1
                        g, r = mt_dims(mt)
                        h0 = h0s.pop(mt)
                        # conv1 [64+1]->[256]; leaky on Pool
                        h1 = spool.tile([128, 2, MT_R], BF16, tag="h1")
                        ps1 = psc1_pool.tile([128, 2, 512], F32, tag="ps1")
                        for c in range(2):
                            nc.tensor.matmul(ps1[:, c, 0:r], w1_t[:, c, :],
                                             h0[:, 0:r], start=True, stop=True)
                        nc.scalar.activation(h1[:, :, 0:r], ps1[:, :, 0:r],
                                             LRELU, bias=0.0, scale=1.0,
                                             alpha=SLOPE)
                        h1s[mt] = h1
                    pump()

                    if 2 <= t < N_TOT + 2:
                        mt = t - 2
                        g, r = mt_dims(mt)
                        h1 = h1s.pop(mt)
                        # conv2 [256]->[256]; bias+leaky on ACT
                        h2 = spool.tile([128, 2, MT_R], BF16, tag="h2")
                        for c in range(2):
                            psf = psb_pool.tile([128, 512], F32, tag="psb")
                            ps = psf[:, 0:MT_R]
                            nc.tensor.matmul(ps[:, 0:r],
                                             w2_t[:, 0, c * 128:(c + 1) * 128],
                                             h1[:, 0, 0:r],
                                             start=True, stop=False)
                            nc.tensor.matmul(ps[:, 0:r],
                                             w2_t[:, 1, c * 128:(c + 1) * 128],
                                             h1[:, 1, 0:r],
                                             start=False, stop=True)
                            nc.scalar.activation(h2[:, c, 0:r], ps[:, 0:r],
                                                 LRELU, bias=b2_t[:, c:c + 1],
                                                 scale=1.0, alpha=SLOPE)
                        h2s[mt] = h2

                    if 3 <= t < N_TOT + 3:
                        mt = t - 3
                        g, r = mt_dims(mt)
                        g0 = mt * MT_G
                        h2 = h2s.pop(mt)
                        # conv3 [256]->[256]; bias+leaky on ACT
                        h3 = spool.tile([128, 2, MT_R], BF16, tag="h3")
                        for c in range(2):
                            psf = psb_pool.tile([128, 512], F32, tag="psb")
                            ps = psf[:, 0:MT_R]
                            nc.tensor.matmul(ps[:, 0:r],
                                             w3_t[:, 0, c * 128:(c + 1) * 128],
                                             h2[:, 0, 0:r],
                                             start=True, stop=False)
                            nc.tensor.matmul(ps[:, 0:r],
                                             w3_t[:, 1, c * 128:(c + 1) * 128],
                                             h2[:, 1, 0:r],
                                             start=False, stop=True)
                            u3 = spool.tile([128, MT_R], BF16, tag=f"u3{c}")
                            v3 = spool.tile([128, MT_R], BF16, tag=f"v3{c}")
                            nc.vector.tensor_scalar(
                                u3[:, 0:r], ps[:, 0:r], b3_t[:, c:c + 1],
                                None, op0=ADD)
                            nc.vector.tensor_scalar(
                                v3[:, 0:r], u3[:, 0:r], SLOPE, None, op0=MULT)
                            nc.vector.tensor_tensor(
                                h3[:, c, 0:r], u3[:, 0:r], v3[:, 0:r], op=MAX)
                        # mean-pool over 17 nodes (sum; 1/17 inside Wh1)
                        # as an add tree: L1-2 on DVE tensor_tensor (2x perf
                        # mode on packed bf16), L3-5 on Pool via stt-adds.
                        # k-plane stride is exactly 30*17, so (k, g) flattens
                        # into one 60-wide dim and every AP stays <= 3D.
                        if mt < N_MT:
                            hv = h3[:, :, 0:r].rearrange(
                                "p k (g s) -> p (k g) s", s=C)
                            KG = 2 * MT_G
                            t8 = spool.tile([128, KG, 8], BF16, tag="t8")
                            t4 = spool.tile([128, KG, 4], BF16, tag="t4")
                            t2 = spool.tile([128, KG, 2], BF16, tag="t2")
                            t1 = spool.tile([128, KG], BF16, tag="t1")
                            nc.gpsimd.tensor_tensor(
                                t8[:], hv[:, :, 0:8], hv[:, :, 8:16], op=ADD)
                            nc.gpsimd.tensor_tensor(
                                t4[:], t8[:, :, 0:4], t8[:, :, 4:8], op=ADD)
                            nc.gpsimd.tensor_tensor(
                                t2[:], t4[:, :, 0:2], t4[:, :, 2:4], op=ADD)
                            nc.gpsimd.tensor_tensor(
                                t1[:], t2[:, :, 0], t2[:, :, 1], op=ADD)
                            nc.gpsimd.tensor_tensor(
                                gt[:, :, g0:g0 + g],
                                t1[:].rearrange("p (k g) -> p k g", k=2),
                                hv[:, :, 16].rearrange(
                                    "p (k g) -> p k g", k=2),
                                op=ADD)
                        else:
                            for k in range(2):
                                hvk = h3[:, k, 0:r].rearrange(
                                    "p (g s) -> p g s", s=C)
                                w8 = spool.tile([128, TAIL_G, 8], BF16,
                                                tag="w8")
                                w4 = spool.tile([128, TAIL_G, 4], BF16,
                                                tag="w4")
                                w2 = spool.tile([128, TAIL_G, 2], BF16,
                                                tag="w2")
                                w1t = spool.tile([128, TAIL_G], BF16,
                                                 tag="w1t")
                                nc.gpsimd.tensor_tensor(
                                    w8[:], hvk[:, :, 0:8], hvk[:, :, 8:16],
                                    op=ADD)
                                nc.gpsimd.tensor_tensor(
                                    w4[:], w8[:, :, 0:4], w8[:, :, 4:8],
                                    op=ADD)
                                nc.gpsimd.tensor_tensor(
                                    w2[:], w4[:, :, 0:2], w4[:, :, 2:4],
                                    op=ADD)
                                nc.gpsimd.tensor_tensor(
                                    w1t[:], w2[:, :, 0], w2[:, :, 1], op=ADD)
                                nc.gpsimd.tensor_tensor(
                                    gt[:, k, g0:g0 + g], w1t[:],
                                    hvk[:, :, 16], op=ADD)
                        # head tiles become eligible one step after
                        # their last reduce was emitted, so the gt columns
                        # are already written when the head matmuls issue
                        while (next_ht + 1) * HT_G <= graphs_done:
                            enqueue_head(next_ht)
                            next_ht += 1
                        graphs_done += g
                    pump()

                while next_ht < BS // HT_G:
                    enqueue_head(next_ht)
                    next_ht += 1
                while pending:
                    pump()

    nc.compile()
    return nc


def _get_nc(reps=1):
    key = ("nc", reps)
    if key not in _CACHE:
        _CACHE[key] = _build(reps)
    return _CACHE[key]


def prep_weights(W_emb, b_emb, W1, b1, W2, b2, W3, b3, Wh1, bh1, Wh2, bh2):
    """Host-side packing of all weight/bias tensors into device layouts."""
    def kchunks(w):
        # [256, out] -> [128, 2, out] (k-chunk as middle axis)
        return np.ascontiguousarray(
            w.reshape(2, 128, w.shape[1]).transpose(1, 0, 2))

    def bcols(b):
        # [256] -> [128, 2] fp32
        return np.ascontiguousarray(b.reshape(2, 128).T.astype(np.float32))

    wemb81 = np.vstack([W_emb, b_emb[None, :]]).astype(BF)         # [81, 64]
    w1c = np.stack(
        [np.vstack([W1[:, c * 128:(c + 1) * 128],
                    b1[None, c * 128:(c + 1) * 128]]) for c in range(2)],
        axis=1).astype(BF)                                          # [65,2,128]
    return {
        "wemb": np.ascontiguousarray(wemb81),
        "w1": np.ascontiguousarray(w1c),
        "w2": kchunks(W2).astype(BF), "b2": bcols(b2),
        "w3": kchunks(W3).astype(BF), "b3": bcols(b3),
        "wh1": kchunks(Wh1 * (1.0 / C)).astype(BF), "bh1": bcols(bh1),
        "wh2": kchunks(Wh2).astype(BF),
        "bh2r": np.ascontiguousarray(
            np.broadcast_to(bh2[None, :], (128, L)).astype(np.float32)),
    }


def prep_x(x):
    """[B, C, T] f32 -> per-core feature-major [81, R] bf16, row 80 = ones."""
    xT = np.ascontiguousarray(x.reshape(B * C, T).T)   # [80, B*C]
    per_core = []
    for i in range(NCORES):
        xc = np.empty((81, R), np.float32)
        xc[0:T] = xT[:, i * R:(i + 1) * R]
        xc[T] = 1.0
        per_core.append(xc.astype(BF))
    return per_core


def _fixup_graph0(x, W_emb, b_emb, W1, b1, W2, b2, W3, b3, Wh1, bh1, Wh2, bh2,
                  src, dst):
    """Exact recompute of batch element 0 with real GCN propagation."""
    deg = np.ones(C, np.float64)
    np.add.at(deg, dst.astype(np.int64), 1.0)
    dinv = 1.0 / np.sqrt(deg)
    A = np.zeros((C, C), np.float64)
    A[np.arange(C), np.arange(C)] = dinv * dinv
    np.add.at(A, (dst.astype(np.int64), src.astype(np.int64)),
              dinv[src.astype(np.int64)] * dinv[dst.astype(np.int64)])

    h = _leaky_np(x[0].astype(np.float64) @ W_emb + b_emb)
    for Wc, bc in [(W1, b1), (W2, b2), (W3, b3)]:
        h = _leaky_np(A @ (h @ Wc) + bc)
    g = h.mean(axis=0)
    return (_leaky_np(g @ Wh1 + bh1) @ Wh2 + bh2).astype(np.float32)


def kernel(x, W_emb, b_emb, W1, b1, W2, b2, W3, b3, Wh1, bh1, Wh2, bh2,
           src, dst):
    x = np.ascontiguousarray(np.asarray(x, np.float32))
    W_emb = np.asarray(W_emb, np.float32)
    b_emb = np.asarray(b_emb, np.float32)
    W1 = np.asarray(W1, np.float32)
    b1 = np.asarray(b1, np.float32)
    W2 = np.asarray(W2, np.float32)
    b2 = np.asarray(b2, np.float32)
    W3 = np.asarray(W3, np.float32)
    b3 = np.asarray(b3, np.float32)
    Wh1 = np.asarray(Wh1, np.float32)
    bh1 = np.asarray(bh1, np.float32)
    Wh2 = np.asarray(Wh2, np.float32)
    bh2 = np.asarray(bh2, np.float32)

    weights = prep_weights(W_emb, b_emb, W1, b1, W2, b2, W3, b3,
                           Wh1, bh1, Wh2, bh2)
    xs = prep_x(x)
    in_maps = []
    for i in range(NCORES):
        m = dict(weights)
        m["x"] = xs[i]
        in_maps.append(m)

    nc = _get_nc()
    res = run_bass_kernel_spmd(nc, in_maps, core_ids=list(range(NCORES)))
    out = np.concatenate([res.results[i]["out"] for i in range(NCORES)], axis=0)

    out[0] = _fixup_graph0(x, W_emb, b_emb, W1, b1, W2, b2, W3, b3,
                           Wh1, bh1, Wh2, bh2, np.asarray(src), np.asarray(dst))
    return out
